# revision 20
# baseline (speedup 1.0000x reference)
"""Trainium2 Bass kernel for nn_MultiHeadedAttention_71425306132929.

Fused QKV projection + RoPE + causal/padding-masked SDPA + output projection.

Sharding: 8 cores = 2 batches x 4 head-groups (4 heads each).  Each core
computes, for its (batch, head-group):
    qkT = (Wq|Wk) @ query[b].T   (fp8 DoubleRow matmuls, head-dim on partitions)
    RoPE on qT/kT: partner dims are 32 partitions apart (natural layout); the
    rotate-half shuffle is 4 small SBUF->SBUF partition-offset DMAs
    scoresT[k,q] per head via K=64 bf16 matmuls (2 heads row-tiled in quadrants)
    causal mask: per-key-block column trim + a PE-seeded -3000 upper triangle
    added into diagonal psum blocks (exp then gives exact 0; no gpsimd selects)
    PT = exp(scoresT/8) written as fp8e4m3 (no max-subtraction: logits O(1))
    padding mask folded into v (zeroed rows) + an all-mask column per key tile
    so the softmax denominator falls out of the same matmul
    ohT = (v|m).T @ PT as fp8 DoubleRow matmuls over 256-key superblocks
    normalize: reciprocal of den rows + f32 PE broadcast matmul + elementwise
    yT_partial = WoutT.T @ ohT as fp8 DoubleRow (row-parallel out-projection)
Host sums the 4 partial yT per batch.

PE work is manually interleaved (projection/out-projection matmuls are pulled
from a filler queue between attention matmuls) to keep the PE continuously
streaming at full pstate.
"""

import os
import sys
from collections import deque

import numpy as np

sys.path.insert(0, "/opt/trn_rl_repo")

import concourse.bass as bass  # noqa: E402
import concourse.bacc as bacc  # noqa: E402
import concourse.tile as tile  # noqa: E402
from concourse import mybir  # noqa: E402

import ml_dtypes  # noqa: E402

BF16 = mybir.dt.bfloat16
F32 = mybir.dt.float32
F8 = mybir.dt.float8e4
DR = mybir.MatmulPerfMode.DoubleRow

B, S, DM, TD, H, HD = 2, 2048, 1024, 1024, 16, 64
NCORES = 8
NH = 4          # heads per core
NKB = S // 128  # 16 key blocks
NQC = S // 512  # 4 query chunks

_CACHED = {}


def build_program():
    nc = bacc.Bacc(None, target_bir_lowering=False)
    qT_d = nc.declare_dram_parameter("qT", [DM, S], BF16, isOutput=False)
    wqk_d = nc.declare_dram_parameter("wqkT", [DM, 512], BF16, isOutput=False)
    wv_d = nc.declare_dram_parameter("wvT", [DM, 256], BF16, isOutput=False)
    cos_d = nc.declare_dram_parameter("cosT", [128, S], BF16, isOutput=False)
    sin_d = nc.declare_dram_parameter("sinT", [128, S], BF16, isOutput=False)
    mkv_d = nc.declare_dram_parameter("maskv", [128, NKB], F32, isOutput=False)
    wo_d = nc.declare_dram_parameter("woT", [256, DM], BF16, isOutput=False)
    tri_d = nc.declare_dram_parameter("tri", [128, 128], BF16, isOutput=False)
    i128_d = nc.declare_dram_parameter("i128", [128, 128], BF16, isOutput=False)
    yT_d = nc.declare_dram_parameter("yT", [DM, S], F32, isOutput=True)
    dden = nc.dram_tensor("den_scratch", [16, 512], F32)

    with tile.TileContext(nc) as tc:
        with (
            tc.tile_pool(name="const", bufs=1) as cpool,
            tc.tile_pool(name="work", bufs=1) as wpool,
            tc.tile_pool(name="rope", bufs=3) as rpool,
            tc.tile_pool(name="pt", bufs=1) as ptpool,
            tc.tile_pool(name="nrm", bufs=2) as npool,
            tc.tile_pool(name="yout", bufs=3) as ypool,
            tc.tile_pool(name="psS", bufs=2, space="PSUM") as psSp,
            tc.tile_pool(name="psO", bufs=1, space="PSUM") as psOp,
            tc.tile_pool(name="psP", bufs=2, space="PSUM") as psPp,
        ):
            qT_sb = cpool.tile([128, 8, S], BF16, tag="qT")
            wqk_sb = cpool.tile([128, 8, 512], BF16, tag="wqk")
            wv_sb = cpool.tile([128, 8, 256], BF16, tag="wv")
            cos_sb = cpool.tile([128, S], BF16, tag="cos")
            sin_sb = cpool.tile([128, S], BF16, tag="sin")
            mkv_sb = cpool.tile([128, NKB], F32, tag="mkv")
            wo_sb = cpool.tile([128, 2, DM], BF16, tag="wo")
            tri_sb = cpool.tile([128, 128], BF16, tag="tri")
            i128_sb = cpool.tile([128, 128], BF16, tag="i128")

            qk_sb = wpool.tile([128, 4, S], BF16, tag="qk")
            # [key-in-block, superblock, ktile, slot, outdim]
            vaug_sb = wpool.tile([128, 8, 2, 4, 128], BF16, tag="vaug")
            ohT_sb = wpool.tile([128, 2, S], BF16, tag="ohT")

            qT_r = qT_d.rearrange("(c p) s -> p c s", p=128)
            yT_r = yT_d.rearrange("(c p) s -> p c s", p=128)

            # --- input DMAs (sync queue), most-urgent first ---
            nc.sync.dma_start(mkv_sb[:], mkv_d[:])
            nc.sync.dma_start(tri_sb[:], tri_d[:])
            nc.sync.dma_start(i128_sb[:], i128_d[:])
            nc.sync.dma_start(wqk_sb[:], wqk_d.rearrange("(c p) s -> p c s", p=128))
            nc.sync.dma_start(qT_sb[:, :, 0:512], qT_r[:, :, 0:512])
            nc.sync.dma_start(cos_sb[:], cos_d[:])
            nc.sync.dma_start(sin_sb[:], sin_d[:])
            nc.sync.dma_start(wv_sb[:], wv_d.rearrange("(c p) s -> p c s", p=128))
            nc.sync.dma_start(wo_sb[:], wo_d.rearrange("(t p) m -> p t m", p=128))

            # --- vaug constant regions: zeros + mask columns ---
            # slots 0,2 (head 0 of each pair): v at cols 0:64, mask col 64,
            # zeros 65:128.  slots 1,3: v at 64:128, mask col 32, zeros 0:64.
            nc.gpsimd.memset(vaug_sb[:, :, :, 0, 65:128], 0.0)
            nc.gpsimd.memset(vaug_sb[:, :, :, 1, 0:64], 0.0)
            nc.gpsimd.memset(vaug_sb[:, :, :, 2, 65:128], 0.0)
            nc.gpsimd.memset(vaug_sb[:, :, :, 3, 0:64], 0.0)
            mkv_col = mkv_sb.rearrange("p (b t o) -> p b t o", t=2, o=1)
            nc.gpsimd.tensor_copy(vaug_sb[:, :, :, 0, 64:65], mkv_col)
            nc.gpsimd.tensor_copy(vaug_sb[:, :, :, 1, 32:33], mkv_col)
            nc.gpsimd.tensor_copy(vaug_sb[:, :, :, 2, 64:65], mkv_col)
            nc.gpsimd.tensor_copy(vaug_sb[:, :, :, 3, 32:33], mkv_col)

            # ---------- emit helpers ----------

            def emit_qk_tail(mt, qn, ps):
                """RoPE: cast, partition-swap DMAs, mul/mul/add"""
                qsl = slice(qn * 512, qn * 512 + 512)
                qkp = rpool.tile([128, 512], BF16, tag="qkp", bufs=3, name="qkp")
                nc.vector.tensor_copy(qkp[:], ps[:])
                shuf = rpool.tile([128, 512], BF16, tag="shuf", bufs=3,
                                  name="shuf")
                for a in range(2):
                    lo = 64 * a
                    nc.sync.dma_start(shuf[lo:lo + 32, :], qkp[lo + 32:lo + 64, :])
                    nc.sync.dma_start(shuf[lo + 32:lo + 64, :], qkp[lo:lo + 32, :])
                t1 = rpool.tile([128, 512], BF16, tag="t1", bufs=3, name="t1")
                nc.vector.tensor_mul(t1[:], qkp[:], cos_sb[:, qsl])
                t2 = rpool.tile([128, 512], BF16, tag="t2", bufs=3, name="t2")
                nc.vector.tensor_mul(t2[:], shuf[:], sin_sb[:, qsl])
                nc.vector.tensor_add(qk_sb[:, mt, qsl], t1[:], t2[:])

            def emit_v_tail(st, ps):
                sbk, kt = st // 2, st % 2
                psv_h = ps[:, 0:256].rearrange("p (h d) -> p h d", h=4)
                msk = mkv_sb[:, st:st + 1]
                nc.vector.tensor_scalar_mul(
                    vaug_sb[:, sbk, kt, 0:4:2, 0:64], psv_h[:, 0:4:2, :], msk)
                nc.vector.tensor_scalar_mul(
                    vaug_sb[:, sbk, kt, 1:4:2, 64:128], psv_h[:, 1:4:2, :], msk)

            def make_prep_fillers(qn):
                """projection work for query chunk qn, as single-mm closures"""
                out = []
                qsl = slice(qn * 512, qn * 512 + 512)
                for mt in (2, 0, 3, 1):  # k pair0, q pair0, k pair1, q pair1
                    box = {}
                    for c in range(0, 8, 2):
                        def mm(mt=mt, c=c, box=box):
                            if c == 0:
                                box["ps"] = psPp.tile([128, 512], F32,
                                                      tag="pj", name="pjqk")
                            for cc in (c, c + 1):
                                nc.tensor.matmul(
                                    box["ps"][:],
                                    lhsT=wqk_sb[:, cc, mt * 128:(mt + 1) * 128],
                                    rhs=qT_sb[:, cc, qsl],
                                    start=(cc == 0), stop=(cc == 7),
                                )
                            if c == 6:
                                emit_qk_tail(mt, qn, box["ps"])
                        out.append(mm)
                for st in range(4 * qn, 4 * qn + 4):
                    box = {}
                    for c in range(0, 8, 2):
                        def mm(st=st, c=c, box=box):
                            if c == 0:
                                box["ps"] = psPp.tile([128, 512], F32,
                                                      tag="pj", name="pjv")
                            for cc in (c, c + 1):
                                nc.tensor.matmul(
                                    box["ps"][:, 0:256],
                                    lhsT=qT_sb[:, cc, st * 128:(st + 1) * 128],
                                    rhs=wv_sb[:, cc, :],
                                    start=(cc == 0), stop=(cc == 7),
                                )
                            if c == 6:
                                emit_v_tail(st, box["ps"])
                        out.append(mm)
                return out

            def make_outproj_fillers(qn):
                qsl = slice(qn * 512, qn * 512 + 512)
                ybox = {}
                out = []
                for mt in range(8):
                    def mm(mt=mt, qsl=qsl, ybox=ybox):
                        ps = psPp.tile([128, 512], F32, tag="pj", name="pjo")
                        for c2 in range(2):
                            nc.tensor.matmul(
                                ps[:],
                                lhsT=wo_sb[:, c2, mt * 128:(mt + 1) * 128],
                                rhs=ohT_sb[:, c2, qsl],
                                start=(c2 == 0), stop=(c2 == 1),
                            )
                        if mt % 2 == 0:
                            ybox["y"] = ypool.tile([128, 2, 512], F32, tag="y",
                                                   name="yst")
                        nc.vector.tensor_copy(ybox["y"][:, mt % 2, :], ps[:])
                        if mt % 2 == 1:
                            nc.sync.dma_start(
                                yT_r[:, mt - 1:mt + 1, qsl], ybox["y"][:])
                    out.append(mm)
                return out

            fillers = deque()

            def drain(n):
                for _ in range(n):
                    if fillers:
                        fillers.popleft()()

            def sc_group(qn, pair, sb, nsb):
                """scores + seed + exp for one superblock, both heads.
                h0/h1 matmuls are interleaved so adjacent instructions sit in
                different PE row-quadrants and stream concurrently."""
                q0 = qn * 512
                qmt, kmt = pair, 2 + pair
                pts = [ptpool.tile([128, 2, 512], BF16, tag="ptF", bufs=6,
                                   name="ptF") for _ in range(2)]
                pss = [psSp.tile([128, 2, 512], F32, tag="psS", name="stps")
                       for _ in range(2)]
                cos_ = []
                for t in range(2):
                    kb = 2 * sb + t
                    co = max(0, kb * 128 - q0)
                    cos_.append(co)
                    diag = kb * 128 >= q0
                    ksl = slice(kb * 128, kb * 128 + 128)
                    for h in range(2):
                        pr = slice(64 * h, 64 * h + 64)
                        nc.tensor.matmul(
                            pss[h][:, t, co:512],
                            lhsT=qk_sb[pr, kmt, ksl],
                            rhs=qk_sb[pr, qmt, q0 + co:q0 + 512],
                            start=True, stop=not diag,
                            skip_group_check=True,
                        )
                    if diag:
                        # add -3000 upper triangle onto the diagonal blocks
                        for h in range(2):
                            nc.tensor.matmul(
                                pss[h][:, t, co:co + 128],
                                lhsT=tri_sb[:],
                                rhs=i128_sb[:],
                                start=False, stop=True,
                                skip_group_check=True,
                            )
                co0, co1 = cos_
                for h in range(2):
                    if co0 == co1:
                        nc.scalar.activation(
                            pts[h][:, :, co0:512], pss[h][:, :, co0:512],
                            mybir.ActivationFunctionType.Exp, scale=0.125)
                    else:
                        nc.scalar.activation(
                            pts[h][:, 0, co0:512], pss[h][:, 0, co0:512],
                            mybir.ActivationFunctionType.Exp, scale=0.125)
                        nc.scalar.activation(
                            pts[h][:, 1, co1:512], pss[h][:, 1, co1:512],
                            mybir.ActivationFunctionType.Exp, scale=0.125)
                return pts

            def av_group(qn, pair, sb, nsb, oT, pts):
                q0 = qn * 512
                for h in range(2):
                    for t in range(2):
                        kb = 2 * sb + t
                        co = max(0, kb * 128 - q0)
                        nc.tensor.matmul(
                            oT[h][:, co:512],
                            lhsT=vaug_sb[:, sb, t, 2 * pair + h, :],
                            rhs=pts[sb][h][:, t, co:512],
                            start=(sb == 0 and t == 0),
                            stop=(sb == nsb - 1 and t == 1),
                            skip_group_check=True,
                        )

            def norm(qn, pair, oT):
                qsl = slice(qn * 512, qn * 512 + 512)
                den2r = npool.tile([33, 512], F32, tag="den2r", name="den2r")
                nc.vector.reciprocal(den2r[0:1, :], oT[0][64:65, :])
                nc.vector.reciprocal(den2r[32:33, :], oT[1][32:33, :])
                osb0 = npool.tile([128, 512], F32, tag="osb0", name="osb0")
                osb1 = npool.tile([128, 512], F32, tag="osb1", name="osb1")
                nc.vector.tensor_copy(osb0[0:64, :], oT[0][0:64, :])
                nc.vector.tensor_copy(osb1[64:128, :], oT[1][64:128, :])
                bc = npool.tile([128, 512], F32, tag="bc", name="bc")
                di = (2 * qn + pair) * 2
                nc.sync.dma_start(dden[di:di + 1, :], den2r[0:1, :])
                nc.sync.dma_start(dden[di + 1:di + 2, :], den2r[32:33, :])
                nc.sync.dma_start(bc[0:64, :],
                                  dden[di:di + 1, :].to_broadcast((64, 512)))
                nc.sync.dma_start(bc[64:128, :],
                                  dden[di + 1:di + 2, :].to_broadcast((64, 512)))
                nc.gpsimd.tensor_mul(ohT_sb[0:64, pair, qsl],
                                     osb0[0:64, :], bc[0:64, :])
                nc.gpsimd.tensor_mul(ohT_sb[64:128, pair, qsl],
                                     osb1[64:128, :], bc[64:128, :])

            def keyloop(qn, pair, per_sb):
                nsb = 2 * qn + 2
                oT = [psOp.tile([128, 512], F32, tag=f"oT{h}", bufs=1,
                                name=f"oT{h}") for h in range(2)]
                pts = {}
                for sb in range(nsb):
                    pts[sb] = sc_group(qn, pair, sb, nsb)
                    if sb >= 1:
                        av_group(qn, pair, sb - 1, nsb, oT, pts)
                        del pts[sb - 1]
                    drain(per_sb)
                av_group(qn, pair, nsb - 1, nsb, oT, pts)
                norm(qn, pair, oT)

            # ---------- main emission ----------
            for f in make_prep_fillers(0):
                f()

            for qn in range(NQC):
                if qn + 1 < NQC:
                    fillers.extend(make_prep_fillers(qn + 1))
                # spread pending fillers over this qc's superblocks; all of
                # them must be emitted before keyloop(qn+1) reads their output
                nsb2 = 2 * (2 * qn + 2)
                per_sb = -(-len(fillers) // nsb2)
                keyloop(qn, 0, per_sb)
                keyloop(qn, 1, per_sb)
                drain(len(fillers))
                fillers.extend(make_outproj_fillers(qn))
            while fillers:
                fillers.popleft()()

    nc.compile()
    return nc


def make_in_maps(query, W_in, W_out, sin_q, cos_q, attn_mask):
    bf = ml_dtypes.bfloat16
    def tob(x):
        return np.ascontiguousarray(x).astype(bf)

    cosT = np.asarray(cos_q, np.float32)[0, 0].T  # [64, S]
    sinT = np.asarray(sin_q, np.float32)[0, 0].T
    sgn = np.where(np.arange(HD) < 32, -1.0, 1.0).astype(np.float32)
    cos2 = np.concatenate([cosT, cosT], 0).astype(bf)          # [128, S]
    sin2 = np.concatenate([sinT * sgn[:, None]] * 2, 0).astype(bf)
    W_in = np.asarray(W_in, np.float32)
    W_out = np.asarray(W_out, np.float32)
    query = np.asarray(query, np.float32)
    attn_mask = np.asarray(attn_mask)

    tri = np.where(np.arange(128)[:, None] < np.arange(128)[None, :],
                   -3000.0, 0.0).astype(bf)
    i128 = np.eye(128, dtype=np.float32).astype(bf)

    in_maps = []
    for c in range(NCORES):
        b, g = c // 4, c % 4
        heads = range(4 * g, 4 * g + 4)
        qrows = np.concatenate([W_in[h * 64:(h + 1) * 64] for h in heads])
        krows = np.concatenate([W_in[TD + h * 64:TD + (h + 1) * 64]
                                for h in heads])
        vrows = np.concatenate([W_in[2 * TD + h * 64:2 * TD + (h + 1) * 64]
                                for h in heads])
        tcols = np.concatenate([np.arange(h * 64, (h + 1) * 64) for h in heads])
        in_maps.append({
            "qT": tob(query[b].T),
            "wqkT": tob(np.concatenate([qrows, krows], 0).T),
            "wvT": tob(vrows.T),
            "cosT": cos2,
            "sinT": sin2,
            "maskv": np.ascontiguousarray(
                attn_mask[b].astype(np.float32).reshape(NKB, 128).T),
            "woT": tob(W_out[:, tcols].T),
            "tri": tri,
            "i128": i128,
        })
    return in_maps


def _ensure_ntff_hook():
    """The image's antenv lacks axon_hooks; supply it so trace=True works."""
    try:
        from antenv.axon_hooks import get_axon_ntff_profile_hook  # noqa: F401
        return
    except ImportError:
        pass
    import types

    if "/root/.axon_site" not in sys.path:
        sys.path.insert(0, "/root/.axon_site")
    from trn_agent_boot.trn_boot import _ntff_profile_via_ctypes

    hook = _ntff_profile_via_ctypes("/opt/axon/libaxon_pjrt.so")
    mod = types.ModuleType("antenv.axon_hooks")
    mod._hook = hook
    mod.get_axon_ntff_profile_hook = lambda: mod._hook
    mod.set_axon_ntff_profile_hook = lambda h: setattr(mod, "_hook", h)
    sys.modules["antenv.axon_hooks"] = mod
    import antenv

    antenv.axon_hooks = mod


def kernel(query, W_in, W_out, sin_q, cos_q, attn_mask):
    if "nc" not in _CACHED:
        _CACHED["nc"] = build_program()
    nc = _CACHED["nc"]
    in_maps = make_in_maps(query, W_in, W_out, sin_q, cos_q, attn_mask)

    from concourse.bass_utils import run_bass_kernel_spmd

    trace = bool(os.environ.get("KERNEL_PROFILE"))
    if trace:
        try:
            _ensure_ntff_hook()
        except Exception as e:  # profiling is best-effort
            print(f"ntff hook unavailable: {e}")
            trace = False
    try:
        res = run_bass_kernel_spmd(nc, in_maps, list(range(NCORES)), trace=trace)
    except Exception:
        if not trace:
            raise
        res = run_bass_kernel_spmd(nc, in_maps, list(range(NCORES)), trace=False)
    _CACHED["last_result"] = res

    y = np.zeros((B, S, DM), np.float32)
    for c in range(NCORES):
        y[c // 4] += res.results[c]["yT"].T
    return y


# revision 27
# speedup vs baseline: 1.4747x; 1.4747x over previous
"""Trainium2 Bass kernel for nn_MultiHeadedAttention_71425306132929.

Fused QKV projection + RoPE + causal/padding-masked SDPA + output projection.

Sharding: 8 cores = 2 batches x 4 head-groups (4 heads each).  Each core
computes, for its (batch, head-group), all in bf16 (fp8 was tried and is too
lossy for the 2e-2 gate: a single e4m3-quantized tensor already costs ~3e-2):
    qkT = (Wq|Wk) @ query[b].T   (head-dim on partitions)
    RoPE on qT/kT: partner dims are 32 partitions apart (natural layout); the
    rotate-half shuffle is 4 small SBUF->SBUF partition-offset DMAs
    scoresT[k,q] per head via K=64 bf16 matmuls (2 heads row-tiled in
    quadrants; adjacent-quadrant matmuls stream concurrently on the PE)
    causal mask: per-key-block column trim + a PE-seeded -3000 upper triangle
    added into diagonal psum blocks (exp then gives exact 0; no gpsimd selects)
    PT = exp(scoresT/8) in bf16 (no max-subtraction: logits are O(1))
    padding mask folded into v (zeroed rows) + an all-mask column per key tile
    so the softmax denominator falls out of the same matmul
    ohT = (v|m).T @ PT accumulated over key blocks, col-trimmed to valid range
    normalize: reciprocal_approx_fast on den rows + f32 broadcast matmul
    yT_partial = WoutT.T @ ohT  (row-parallel out-projection)
Host sums the 4 partial yT per batch.

PE work is manually interleaved (projection/out-projection matmuls are pulled
from a filler queue between attention matmuls) to keep the PE continuously
streaming at full pstate.
"""

import os
import sys
from collections import deque

import numpy as np

sys.path.insert(0, "/opt/trn_rl_repo")

import concourse.bass as bass  # noqa: E402
import concourse.bacc as bacc  # noqa: E402
import concourse.tile as tile  # noqa: E402
from concourse import mybir  # noqa: E402

import ml_dtypes  # noqa: E402

BF16 = mybir.dt.bfloat16
F32 = mybir.dt.float32
F8 = mybir.dt.float8e4
DR = mybir.MatmulPerfMode.DoubleRow

B, S, DM, TD, H, HD = 2, 2048, 1024, 1024, 16, 64
NCORES = 8
NH = 4          # heads per core
NKB = S // 128  # 16 key blocks
NQC = S // 512  # 4 query chunks

_CACHED = {}


def build_program():
    nc = bacc.Bacc(None, target_bir_lowering=False)
    qT_d = nc.declare_dram_parameter("qT", [DM, S], BF16, isOutput=False)
    wqk_d = nc.declare_dram_parameter("wqkT", [DM, 512], BF16, isOutput=False)
    wv_d = nc.declare_dram_parameter("wvT", [DM, 256], BF16, isOutput=False)
    cos_d = nc.declare_dram_parameter("cosT", [128, S], BF16, isOutput=False)
    sin_d = nc.declare_dram_parameter("sinT", [128, S], BF16, isOutput=False)
    mkv_d = nc.declare_dram_parameter("maskv", [128, NKB], F32, isOutput=False)
    wo_d = nc.declare_dram_parameter("woT", [256, DM], BF16, isOutput=False)
    tri_d = nc.declare_dram_parameter("tri", [128, 128], BF16, isOutput=False)
    i128_d = nc.declare_dram_parameter("i128", [128, 128], BF16, isOutput=False)
    sel2_d = nc.declare_dram_parameter("sel2", [33, 128], F32, isOutput=False)
    yT_d = nc.declare_dram_parameter("yT", [DM, S], F32, isOutput=True)

    with tile.TileContext(nc) as tc:
        with (
            tc.tile_pool(name="const", bufs=1) as cpool,
            tc.tile_pool(name="work", bufs=1) as wpool,
            tc.tile_pool(name="rope", bufs=3) as rpool,
            tc.tile_pool(name="pt", bufs=1) as ptpool,
            tc.tile_pool(name="nrm", bufs=2) as npool,
            tc.tile_pool(name="yout", bufs=3) as ypool,
            tc.tile_pool(name="psS", bufs=2, space="PSUM") as psSp,
            tc.tile_pool(name="psO", bufs=1, space="PSUM") as psOp,
            tc.tile_pool(name="psP", bufs=2, space="PSUM") as psPp,
        ):
            qT_sb = cpool.tile([128, 8, S], BF16, tag="qT")
            wqk_sb = cpool.tile([128, 8, 512], BF16, tag="wqk")
            wv_sb = cpool.tile([128, 8, 256], BF16, tag="wv")
            cos_sb = cpool.tile([128, S], BF16, tag="cos")
            sin_sb = cpool.tile([128, S], BF16, tag="sin")
            mkv_sb = cpool.tile([128, NKB], F32, tag="mkv")
            wo_sb = cpool.tile([128, 2, DM], BF16, tag="wo")
            tri_sb = cpool.tile([128, 128], BF16, tag="tri")
            i128_sb = cpool.tile([128, 128], BF16, tag="i128")
            sel2_sb = cpool.tile([33, 128], F32, tag="sel2")
            den2r_tiles = [cpool.tile([33, 512], F32, tag=f"den2r{j}",
                                      name=f"den2r{j}") for j in range(2)]
            dst_tiles = [cpool.tile([33, 512], F32, tag=f"dst{j}",
                                    name=f"dst{j}") for j in range(2)]

            qk_sb = wpool.tile([128, 4, S], BF16, tag="qk")
            # [key-in-block, superblock, ktile, slot, outdim]
            vaug_sb = wpool.tile([128, 8, 2, 4, 128], BF16, tag="vaug")
            ohT_sb = wpool.tile([128, 2, S], BF16, tag="ohT")

            qT_r = qT_d.rearrange("(c p) s -> p c s", p=128)
            yT_r = yT_d.rearrange("(c p) s -> p c s", p=128)

            # --- input DMAs (sync queue), most-urgent first ---
            nc.sync.dma_start(mkv_sb[:], mkv_d[:])
            nc.sync.dma_start(tri_sb[:], tri_d[:])
            nc.sync.dma_start(i128_sb[:], i128_d[:])
            nc.sync.dma_start(sel2_sb[:], sel2_d[:])
            for j in range(2):
                nc.gpsimd.memset(den2r_tiles[j][0:33, :], 1.0)
                nc.gpsimd.memset(dst_tiles[j][0:33, :], 1.0)
            nc.sync.dma_start(wqk_sb[:], wqk_d.rearrange("(c p) s -> p c s", p=128))
            nc.sync.dma_start(qT_sb[:, :, 0:512], qT_r[:, :, 0:512])
            nc.sync.dma_start(cos_sb[:], cos_d[:])
            nc.sync.dma_start(sin_sb[:], sin_d[:])
            nc.sync.dma_start(wv_sb[:], wv_d.rearrange("(c p) s -> p c s", p=128))
            nc.sync.dma_start(wo_sb[:], wo_d.rearrange("(t p) m -> p t m", p=128))

            # --- vaug constant regions: zeros + mask columns ---
            # slots 0,2 (head 0 of each pair): v at cols 0:64, mask col 64,
            # zeros 65:128.  slots 1,3: v at 64:128, mask col 32, zeros 0:64.
            nc.gpsimd.memset(vaug_sb[:, :, :, 0, 65:128], 0.0)
            nc.gpsimd.memset(vaug_sb[:, :, :, 1, 0:64], 0.0)
            nc.gpsimd.memset(vaug_sb[:, :, :, 2, 65:128], 0.0)
            nc.gpsimd.memset(vaug_sb[:, :, :, 3, 0:64], 0.0)
            mkv_col = mkv_sb.rearrange("p (b t o) -> p b t o", t=2, o=1)
            nc.gpsimd.tensor_copy(vaug_sb[:, :, :, 0, 64:65], mkv_col)
            nc.gpsimd.tensor_copy(vaug_sb[:, :, :, 1, 32:33], mkv_col)
            nc.gpsimd.tensor_copy(vaug_sb[:, :, :, 2, 64:65], mkv_col)
            nc.gpsimd.tensor_copy(vaug_sb[:, :, :, 3, 32:33], mkv_col)

            # ---------- emit helpers ----------

            def emit_qk_tail(mt, qn, ps):
                """RoPE: cast, partition-swap DMAs, mul/mul/add"""
                qsl = slice(qn * 512, qn * 512 + 512)
                qkp = rpool.tile([128, 512], BF16, tag="qkp", bufs=3, name="qkp")
                nc.vector.tensor_copy(qkp[:], ps[:])
                shuf = rpool.tile([128, 512], BF16, tag="shuf", bufs=3,
                                  name="shuf")
                for a in range(2):
                    lo = 64 * a
                    nc.sync.dma_start(shuf[lo:lo + 32, :], qkp[lo + 32:lo + 64, :])
                    nc.sync.dma_start(shuf[lo + 32:lo + 64, :], qkp[lo:lo + 32, :])
                t1 = rpool.tile([128, 512], BF16, tag="t1", bufs=3, name="t1")
                nc.vector.tensor_mul(t1[:], qkp[:], cos_sb[:, qsl])
                t2 = rpool.tile([128, 512], BF16, tag="t2", bufs=3, name="t2")
                nc.gpsimd.tensor_mul(t2[:], shuf[:], sin_sb[:, qsl])
                nc.gpsimd.tensor_add(qk_sb[:, mt, qsl], t1[:], t2[:])

            def emit_v_tail(st, ps):
                sbk, kt = st // 2, st % 2
                psv_h = ps[:, 0:256].rearrange("p (h d) -> p h d", h=4)
                msk = mkv_sb[:, st:st + 1]
                nc.vector.tensor_scalar_mul(
                    vaug_sb[:, sbk, kt, 0:4:2, 0:64], psv_h[:, 0:4:2, :], msk)
                nc.vector.tensor_scalar_mul(
                    vaug_sb[:, sbk, kt, 1:4:2, 64:128], psv_h[:, 1:4:2, :], msk)

            def make_prep_fillers(qn):
                """projection work for query chunk qn, as single-mm closures"""
                out = []
                qsl = slice(qn * 512, qn * 512 + 512)
                for mt in (2, 0, 3, 1):  # k pair0, q pair0, k pair1, q pair1
                    box = {}
                    for c in range(0, 8, 2):
                        def mm(mt=mt, c=c, box=box):
                            if c == 0:
                                box["ps"] = psPp.tile([128, 512], F32,
                                                      tag="pj", name="pjqk")
                            for cc in (c, c + 1):
                                nc.tensor.matmul(
                                    box["ps"][:],
                                    lhsT=wqk_sb[:, cc, mt * 128:(mt + 1) * 128],
                                    rhs=qT_sb[:, cc, qsl],
                                    start=(cc == 0), stop=(cc == 7),
                                )
                            if c == 6:
                                emit_qk_tail(mt, qn, box["ps"])
                        out.append(mm)
                for st in range(4 * qn, 4 * qn + 4):
                    box = {}
                    for c in range(0, 8, 2):
                        def mm(st=st, c=c, box=box):
                            if c == 0:
                                box["ps"] = psPp.tile([128, 512], F32,
                                                      tag="pj", name="pjv")
                            for cc in (c, c + 1):
                                nc.tensor.matmul(
                                    box["ps"][:, 0:256],
                                    lhsT=qT_sb[:, cc, st * 128:(st + 1) * 128],
                                    rhs=wv_sb[:, cc, :],
                                    start=(cc == 0), stop=(cc == 7),
                                )
                            if c == 6:
                                emit_v_tail(st, box["ps"])
                        out.append(mm)
                return out

            def make_outproj_fillers(qn):
                qsl = slice(qn * 512, qn * 512 + 512)
                ybox = {}
                out = []
                for mt in range(8):
                    def mm(mt=mt, qsl=qsl, ybox=ybox):
                        ps = psPp.tile([128, 512], F32, tag="pj", name="pjo")
                        for c2 in range(2):
                            nc.tensor.matmul(
                                ps[:],
                                lhsT=wo_sb[:, c2, mt * 128:(mt + 1) * 128],
                                rhs=ohT_sb[:, c2, qsl],
                                start=(c2 == 0), stop=(c2 == 1),
                            )
                        if mt % 2 == 0:
                            ybox["y"] = ypool.tile([128, 2, 512], F32, tag="y",
                                                   name="yst")
                        nc.vector.tensor_copy(ybox["y"][:, mt % 2, :], ps[:])
                        if mt % 2 == 1:
                            nc.sync.dma_start(
                                yT_r[:, mt - 1:mt + 1, qsl], ybox["y"][:])
                    out.append(mm)
                return out

            fillers = deque()

            def drain(n):
                for _ in range(n):
                    if fillers:
                        fillers.popleft()()

            def sc_group(qn, pair, sb, nsb):
                """scores + seed + exp for one superblock, both heads.
                h0/h1 matmuls are interleaved so adjacent instructions sit in
                different PE row-quadrants and stream concurrently."""
                q0 = qn * 512
                qmt, kmt = pair, 2 + pair
                pts = [ptpool.tile([128, 2, 512], BF16, tag="ptF", bufs=6,
                                   name="ptF") for _ in range(2)]
                pss = [psSp.tile([128, 2, 512], F32, tag="psS", name="stps")
                       for _ in range(2)]
                cos_ = []
                for t in range(2):
                    kb = 2 * sb + t
                    co = max(0, kb * 128 - q0)
                    cos_.append(co)
                    diag = kb * 128 >= q0
                    ksl = slice(kb * 128, kb * 128 + 128)
                    for h in range(2):
                        pr = slice(64 * h, 64 * h + 64)
                        nc.tensor.matmul(
                            pss[h][:, t, co:512],
                            lhsT=qk_sb[pr, kmt, ksl],
                            rhs=qk_sb[pr, qmt, q0 + co:q0 + 512],
                            start=True, stop=not diag,
                            skip_group_check=True,
                        )
                    if diag:
                        # add -3000 upper triangle onto the diagonal blocks
                        for h in range(2):
                            nc.tensor.matmul(
                                pss[h][:, t, co:co + 128],
                                lhsT=tri_sb[:],
                                rhs=i128_sb[:],
                                start=False, stop=True,
                                skip_group_check=True,
                            )
                co0, co1 = cos_
                for h in range(2):
                    if co0 == co1:
                        nc.scalar.activation(
                            pts[h][:, :, co0:512], pss[h][:, :, co0:512],
                            mybir.ActivationFunctionType.Exp, scale=0.125)
                    else:
                        nc.scalar.activation(
                            pts[h][:, 0, co0:512], pss[h][:, 0, co0:512],
                            mybir.ActivationFunctionType.Exp, scale=0.125)
                        nc.scalar.activation(
                            pts[h][:, 1, co1:512], pss[h][:, 1, co1:512],
                            mybir.ActivationFunctionType.Exp, scale=0.125)
                return pts

            def av_group(qn, pair, sb, nsb, oT, pts):
                q0 = qn * 512
                for h in range(2):
                    for t in range(2):
                        kb = 2 * sb + t
                        co = max(0, kb * 128 - q0)
                        nc.tensor.matmul(
                            oT[h][:, co:512],
                            lhsT=vaug_sb[:, sb, t, 2 * pair + h, :],
                            rhs=pts[sb][h][:, t, co:512],
                            start=(sb == 0 and t == 0),
                            stop=(sb == nsb - 1 and t == 1),
                            skip_group_check=True,
                        )

            def norm(qn, pair, oT):
                qsl = slice(qn * 512, qn * 512 + 512)
                par = (2 * qn + pair) % 2
                den2r, dst = den2r_tiles[par], dst_tiles[par]
                # custom-DVE ops can't read PSUM and are only correct at
                # partition base 0 on HW: stage den rows, one recip over 0:33
                nc.vector.tensor_copy(dst[0:1, :], oT[0][64:65, :])
                nc.vector.tensor_copy(dst[32:33, :], oT[1][32:33, :])
                nc.vector.reciprocal_approx_fast(den2r[0:33, :], dst[0:33, :])
                # partition-broadcast 1/den via an f32r matmul (f32 data at
                # bf16 stream rate); rows 1:32 of den2r are constant 1.0 and
                # hit zero weights in sel2
                bc = psPp.tile([128, 512], F32, tag="pj", name="bc")
                nc.tensor.matmul(bc[:], lhsT=sel2_sb[:], rhs=den2r[:],
                                 start=True, stop=True)
                bc_sb = npool.tile([128, 512], F32, tag="bc", name="bc_sb")
                nc.vector.tensor_copy(bc_sb[:], bc[:])
                nc.vector.tensor_mul(ohT_sb[0:64, pair, qsl],
                                     oT[0][0:64, :], bc_sb[0:64, :])
                nc.vector.tensor_mul(ohT_sb[64:128, pair, qsl],
                                     oT[1][64:128, :], bc_sb[64:128, :])

            def keyloop(qn, pair, per_sb):
                nsb = 2 * qn + 2
                oT = [psOp.tile([128, 512], F32, tag=f"oT{h}", bufs=1,
                                name=f"oT{h}") for h in range(2)]
                pts = {}
                for sb in range(nsb):
                    pts[sb] = sc_group(qn, pair, sb, nsb)
                    if sb >= 1:
                        av_group(qn, pair, sb - 1, nsb, oT, pts)
                        del pts[sb - 1]
                    drain(per_sb)
                av_group(qn, pair, nsb - 1, nsb, oT, pts)
                norm(qn, pair, oT)

            # ---------- main emission ----------
            for f in make_prep_fillers(0):
                f()

            for qn in range(NQC):
                if qn + 1 < NQC:
                    qsl_n = slice((qn + 1) * 512, (qn + 1) * 512 + 512)
                    nc.sync.dma_start(qT_sb[:, :, qsl_n], qT_r[:, :, qsl_n])
                    fillers.extend(make_prep_fillers(qn + 1))
                # spread pending fillers over this qc's superblocks; all of
                # them must be emitted before keyloop(qn+1) reads their output
                nsb2 = 2 * (2 * qn + 2)
                per_sb = -(-len(fillers) // nsb2)
                keyloop(qn, 0, per_sb)
                keyloop(qn, 1, per_sb)
                drain(len(fillers))
                fillers.extend(make_outproj_fillers(qn))
            while fillers:
                fillers.popleft()()

    nc.compile()
    return nc


def make_in_maps(query, W_in, W_out, sin_q, cos_q, attn_mask):
    bf = ml_dtypes.bfloat16
    def tob(x):
        return np.ascontiguousarray(x).astype(bf)

    cosT = np.asarray(cos_q, np.float32)[0, 0].T  # [64, S]
    sinT = np.asarray(sin_q, np.float32)[0, 0].T
    sgn = np.where(np.arange(HD) < 32, -1.0, 1.0).astype(np.float32)
    cos2 = np.concatenate([cosT, cosT], 0).astype(bf)          # [128, S]
    sin2 = np.concatenate([sinT * sgn[:, None]] * 2, 0).astype(bf)
    W_in = np.asarray(W_in, np.float32)
    W_out = np.asarray(W_out, np.float32)
    query = np.asarray(query, np.float32)
    attn_mask = np.asarray(attn_mask)

    tri = np.where(np.arange(128)[:, None] < np.arange(128)[None, :],
                   -3000.0, 0.0).astype(bf)
    i128 = np.eye(128, dtype=np.float32).astype(bf)
    sel2 = np.zeros((33, 128), np.float32)
    sel2[0, 0:64] = 1.0
    sel2[32, 64:128] = 1.0

    in_maps = []
    for c in range(NCORES):
        b, g = c // 4, c % 4
        heads = range(4 * g, 4 * g + 4)
        qrows = np.concatenate([W_in[h * 64:(h + 1) * 64] for h in heads])
        krows = np.concatenate([W_in[TD + h * 64:TD + (h + 1) * 64]
                                for h in heads])
        vrows = np.concatenate([W_in[2 * TD + h * 64:2 * TD + (h + 1) * 64]
                                for h in heads])
        tcols = np.concatenate([np.arange(h * 64, (h + 1) * 64) for h in heads])
        in_maps.append({
            "qT": tob(query[b].T),
            "wqkT": tob(np.concatenate([qrows, krows], 0).T),
            "wvT": tob(vrows.T),
            "cosT": cos2,
            "sinT": sin2,
            "maskv": np.ascontiguousarray(
                attn_mask[b].astype(np.float32).reshape(NKB, 128).T),
            "woT": tob(W_out[:, tcols].T),
            "tri": tri,
            "i128": i128,
            "sel2": sel2,
        })
    return in_maps


def _ensure_ntff_hook():
    """The image's antenv lacks axon_hooks; supply it so trace=True works."""
    try:
        from antenv.axon_hooks import get_axon_ntff_profile_hook  # noqa: F401
        return
    except ImportError:
        pass
    import types

    if "/root/.axon_site" not in sys.path:
        sys.path.insert(0, "/root/.axon_site")
    from trn_agent_boot.trn_boot import _ntff_profile_via_ctypes

    hook = _ntff_profile_via_ctypes("/opt/axon/libaxon_pjrt.so")
    mod = types.ModuleType("antenv.axon_hooks")
    mod._hook = hook
    mod.get_axon_ntff_profile_hook = lambda: mod._hook
    mod.set_axon_ntff_profile_hook = lambda h: setattr(mod, "_hook", h)
    sys.modules["antenv.axon_hooks"] = mod
    import antenv

    antenv.axon_hooks = mod


def kernel(query, W_in, W_out, sin_q, cos_q, attn_mask):
    if "nc" not in _CACHED:
        _CACHED["nc"] = build_program()
    nc = _CACHED["nc"]
    in_maps = make_in_maps(query, W_in, W_out, sin_q, cos_q, attn_mask)

    from concourse.bass_utils import run_bass_kernel_spmd

    trace = bool(os.environ.get("KERNEL_PROFILE"))
    if trace:
        try:
            _ensure_ntff_hook()
        except Exception as e:  # profiling is best-effort
            print(f"ntff hook unavailable: {e}")
            trace = False
    try:
        res = run_bass_kernel_spmd(nc, in_maps, list(range(NCORES)), trace=trace)
    except Exception:
        if not trace:
            raise
        res = run_bass_kernel_spmd(nc, in_maps, list(range(NCORES)), trace=False)
    _CACHED["last_result"] = res

    y = np.zeros((B, S, DM), np.float32)
    for c in range(NCORES):
        y[c // 4] += res.results[c]["yT"].T
    return y
